# revision 8
# baseline (speedup 1.0000x reference)
"""GQA attention (RoPE, causal) + o_proj on 8 Trainium2 NeuronCores.

Sharding: 8 cores = 2 batches (DP) x 4 kv-head-pairs (TP over GQA groups).
Per core: hs[batch] [S,D], Wq slice [D,512] (8 q heads), Wk/Wv slice [D,128]
(2 kv heads), Wo slice [512,D]. Core computes its heads' attention and a
partial o_proj output [S,D]; host sums 4 partials per batch.

Kernel dataflow (per core, fp32 storage / fp32r matmuls):
  1. hs -> PE-transpose -> hsT ; q/k/v = hsT.T @ W (natural layout)
  2. RoPE in natural layout (host passes replicated/sign-folded trig tables)
  3. PE-transpose q,k -> qT [hd,s], k2T (kv head replicated on both
     64-partition halves for 2-head row-packed score matmuls)
  4. scores^T[k,q] = k2T.T @ qT per 128k x 512q block, causal blocks only,
     two heads packed via tile_position row groups
  5. P^T = exp(scores^T) (ACT), 0/1 causal mask multiplied on diag blocks
  6. A^T_aug = [V|1].T @ P^T accumulated over k tiles in PSUM; row 64 is the
     softmax denominator; normalize A^T rows via reciprocal+partition_broadcast
  7. y = A @ Wo via lhsT=A^T, accumulate 4 feature tiles, DMA out
"""
import sys
import numpy as np

sys.path.insert(0, "/opt/trn_rl_repo")

B, S, D = 2, 2048, 2048
H, KVH, HD = 32, 8, 64
SCALE = HD ** -0.5
P = 128


def build_nc(S=S, D=D, LQ=8, LKV=2, HD=64):
    import concourse.bass as bass
    import concourse.bacc as bacc
    import concourse.mybir as mybir
    from concourse import tile
    from concourse.masks import make_identity

    f32 = mybir.dt.float32
    f32r = mybir.dt.float32r

    QF = LQ * HD          # local q features (512)
    KF = LKV * HD         # local kv features (128)
    FT = QF // P          # q feature tiles = head pairs (4)
    DT = D // P           # contraction tiles (16)
    ST = S // P           # sequence tiles (16)
    NJ = S // 512         # q supertiles (4)
    GRP = LQ // LKV       # q heads per kv head (4)
    VW = HD + 1           # v + ones column (65)

    nc = bacc.Bacc(None, target_bir_lowering=False)
    hs = nc.declare_dram_parameter("hs", [S, D], f32r, isOutput=False)
    wq = nc.declare_dram_parameter("wq", [D, QF], f32r, isOutput=False)
    wkv = nc.declare_dram_parameter("wkv", [D, 2 * KF], f32r, isOutput=False)
    wo = nc.declare_dram_parameter("wo", [QF, D], f32r, isOutput=False)
    cosq = nc.declare_dram_parameter("cosq", [S, QF], f32, isOutput=False)
    sinq = nc.declare_dram_parameter("sinq", [S, QF], f32, isOutput=False)
    cosk = nc.declare_dram_parameter("cosk", [S, KF], f32, isOutput=False)
    sink = nc.declare_dram_parameter("sink", [S, KF], f32, isOutput=False)
    out = nc.declare_dram_parameter("out", [S, D], f32, isOutput=True)

    hs_t = hs.rearrange("(st p) d -> p st d", p=P)
    wq_t = wq.rearrange("(dt p) f -> p dt f", p=P)
    wkv_t = wkv.rearrange("(dt p) f -> p dt f", p=P)
    wo_t = wo.rearrange("(ft p) d -> p ft d", p=P)
    out_t = out.rearrange("(st p) d -> p st d", p=P)

    with tile.TileContext(nc) as tc:
        with tc.tile_pool(name="persist", bufs=1) as persist:
            ident = persist.tile([P, P], f32r)
            mask01 = persist.tile([P, 4, 512], f32r)
            qT = persist.tile([P, FT, S], f32r)
            k2T = persist.tile([P, LKV, S], f32r)
            vaug = persist.tile([P, ST, LKV * VW], f32r)
            with tc.tile_pool(name="init", bufs=1) as init_p:
                ident_f = init_p.tile([P, P], f32)
                make_identity(nc, ident_f[:])
                nc.vector.tensor_copy(ident[:], ident_f[:])
                # mask01[kr, m, qc] = 1 where visible (qc >= 128*m+kr) else 0
                mask_f = init_p.tile([P, 4, 512], f32)
                for m in range(4):
                    nc.gpsimd.memset(mask_f[:, m, :], 1.0)
                    nc.gpsimd.affine_select(
                        out=mask_f[:, m, :], in_=mask_f[:, m, :],
                        compare_op=mybir.AluOpType.is_ge, fill=0.0,
                        base=-P * m, pattern=[[1, 512]], channel_multiplier=-1,
                    )
                nc.vector.tensor_copy(mask01[:], mask_f[:])
                ones_f = init_p.tile([P, 1], f32)
                nc.gpsimd.memset(ones_f[:], 1.0)
                for lkv in range(LKV):
                    for st_i in range(ST):
                        nc.vector.tensor_copy(
                            vaug[:, st_i, lkv * VW + HD:lkv * VW + HD + 1],
                            ones_f[:])

            # ---------------- phase 1: projections + RoPE + transposes ----
            with (
                tc.tile_pool(name="wq_p", bufs=1) as wq_p,
                tc.tile_pool(name="wkv_p", bufs=1) as wkv_p,
                tc.tile_pool(name="stage", bufs=3) as stage_p,
                tc.tile_pool(name="hsT", bufs=2) as hsT_p,
                tc.tile_pool(name="trig", bufs=3) as trig_p,
                tc.tile_pool(name="rope", bufs=2) as rope_p,
                tc.tile_pool(name="ps_tr", bufs=4, space="PSUM") as ps_tr,
                tc.tile_pool(name="ps_q", bufs=2, space="PSUM") as ps_q,
                tc.tile_pool(name="ps_kv", bufs=2, space="PSUM") as ps_kv,
            ):
                wq_sb = wq_p.tile([P, DT, QF], f32r)
                nc.sync.dma_start(out=wq_sb[:], in_=wq_t)
                wkv_sb = wkv_p.tile([P, DT, 2 * KF], f32r)
                nc.sync.dma_start(out=wkv_sb[:], in_=wkv_t)

                for blk in range(S // 256):
                    hsT = hsT_p.tile([P, DT, 256], f32r, tag="hsT")
                    for j in range(2):
                        si = blk * 2 + j
                        st_t = stage_p.tile([P, D], f32r, tag="stage")
                        nc.sync.dma_start(out=st_t[:], in_=hs_t[:, si, :])
                        for dt in range(DT):
                            pt = ps_tr.tile([P, P], f32r, tag="ptr")
                            nc.tensor.transpose(
                                pt[:], st_t[:, dt * P:(dt + 1) * P], ident[:])
                            nc.vector.tensor_copy(
                                hsT[:, dt, j * P:(j + 1) * P], pt[:])
                    for j in range(2):
                        si = blk * 2 + j
                        # ---- q = hs @ Wq (natural), RoPE, transpose ----
                        psq = ps_q.tile([P, QF], f32, tag="psq")
                        for dt in range(DT):
                            nc.tensor.matmul(
                                psq[:], hsT[:, dt, j * P:(j + 1) * P],
                                wq_sb[:, dt, :],
                                start=(dt == 0), stop=(dt == DT - 1))
                        cq = trig_p.tile([P, QF], f32, tag="cq")
                        nc.sync.dma_start(
                            out=cq[:], in_=cosq[si * P:(si + 1) * P, :])
                        sq = trig_p.tile([P, QF], f32, tag="sq")
                        nc.sync.dma_start(
                            out=sq[:], in_=sinq[si * P:(si + 1) * P, :])
                        qrot = rope_p.tile([P, QF], f32r, tag="qrot")
                        qtmp = rope_p.tile([P, QF], f32, tag="qtmp")
                        rsh = rope_p.tile([P, QF], f32, tag="rsh")
                        psq_v = psq[:].rearrange("p (h t x) -> p h t x", t=2, x=32)
                        rsh_v = rsh[:].rearrange("p (h t x) -> p h t x", t=2, x=32)
                        nc.vector.tensor_copy(rsh_v[:, :, 0, :], psq_v[:, :, 1, :])
                        nc.vector.tensor_copy(rsh_v[:, :, 1, :], psq_v[:, :, 0, :])
                        nc.vector.tensor_mul(qtmp[:], psq[:], cq[:])
                        nc.vector.tensor_mul(rsh[:], rsh[:], sq[:])
                        nc.vector.tensor_add(qrot[:], qtmp[:], rsh[:])
                        for ft in range(FT):
                            pt = ps_tr.tile([P, P], f32r, tag="ptr")
                            nc.tensor.transpose(
                                pt[:], qrot[:, ft * P:(ft + 1) * P], ident[:])
                            nc.scalar.copy(qT[:, ft, si * P:(si + 1) * P], pt[:])
                        # ---- k/v ----
                        pskv = ps_kv.tile([P, 2 * KF], f32, tag="pskv")
                        for dt in range(DT):
                            nc.tensor.matmul(
                                pskv[:], hsT[:, dt, j * P:(j + 1) * P],
                                wkv_sb[:, dt, :],
                                start=(dt == 0), stop=(dt == DT - 1))
                        ck = trig_p.tile([P, KF], f32, tag="ck")
                        nc.sync.dma_start(
                            out=ck[:], in_=cosk[si * P:(si + 1) * P, :])
                        sk = trig_p.tile([P, KF], f32, tag="sk")
                        nc.sync.dma_start(
                            out=sk[:], in_=sink[si * P:(si + 1) * P, :])
                        krot = rope_p.tile([P, KF], f32r, tag="krot")
                        ktmp = rope_p.tile([P, KF], f32, tag="ktmp")
                        krsh = rope_p.tile([P, KF], f32, tag="krsh")
                        psk_v = pskv[:, 0:KF].rearrange(
                            "p (h t x) -> p h t x", t=2, x=32)
                        krsh_v = krsh[:].rearrange("p (h t x) -> p h t x", t=2, x=32)
                        nc.vector.tensor_copy(krsh_v[:, :, 0, :], psk_v[:, :, 1, :])
                        nc.vector.tensor_copy(krsh_v[:, :, 1, :], psk_v[:, :, 0, :])
                        nc.vector.tensor_mul(ktmp[:], pskv[:, 0:KF], ck[:])
                        nc.vector.tensor_mul(krsh[:], krsh[:], sk[:])
                        nc.vector.tensor_add(krot[:], ktmp[:], krsh[:])
                        for lkv in range(LKV):
                            pt = ps_tr.tile([P, P], f32r, tag="ptr")
                            nc.tensor.transpose(
                                pt[0:HD, :],
                                krot[:, lkv * HD:(lkv + 1) * HD], ident[:])
                            nc.scalar.copy(
                                k2T[0:HD, lkv, si * P:(si + 1) * P], pt[0:HD, :])
                            nc.scalar.copy(
                                k2T[HD:P, lkv, si * P:(si + 1) * P], pt[0:HD, :])
                            nc.scalar.copy(
                                vaug[:, si, lkv * VW:lkv * VW + HD],
                                pskv[:, KF + lkv * HD:KF + (lkv + 1) * HD])

            # ---------------- phase 2+3: attention + o_proj ---------------
            with (
                tc.tile_pool(name="wo_p", bufs=1) as wo_p,
                tc.tile_pool(name="pt_p", bufs=4) as pt_p,
                tc.tile_pool(name="aT_p", bufs=2) as aT_p,
                tc.tile_pool(name="bc_p", bufs=2) as bc_p,
                tc.tile_pool(name="rd_p", bufs=2) as rd_p,
                tc.tile_pool(name="y_p", bufs=3) as y_p,
                tc.tile_pool(name="ps_s", bufs=4, space="PSUM") as ps_s,
                tc.tile_pool(name="ps_a", bufs=2, space="PSUM") as ps_a,
                tc.tile_pool(name="ps_y", bufs=2, space="PSUM") as ps_y,
            ):
                wo_sb = wo_p.tile([P, FT, D], f32r)
                nc.sync.dma_start(out=wo_sb[:], in_=wo_t)

                for J in range(NJ):
                    aT = aT_p.tile([P, FT, 512], f32r, tag="aT")
                    nkt = 4 * J + 4
                    for t in range(FT):          # head pair (2t, 2t+1)
                        lkv = (2 * t) // GRP
                        psa0 = ps_a.tile([VW, 512], f32, tag="psa")
                        psa1 = ps_a.tile([VW, 512], f32, tag="psa")
                        for kt in range(nkt):
                            pss0 = ps_s.tile([P, 512], f32, tag="pss")
                            pss1 = ps_s.tile([P, 512], f32, tag="pss")
                            nc.tensor.matmul(
                                pss0[:],
                                k2T[0:HD, lkv, kt * P:(kt + 1) * P],
                                qT[0:HD, t, J * 512:(J + 1) * 512],
                                start=True, stop=True, tile_position=(0, 0))
                            nc.tensor.matmul(
                                pss1[:],
                                k2T[HD:P, lkv, kt * P:(kt + 1) * P],
                                qT[HD:P, t, J * 512:(J + 1) * 512],
                                start=True, stop=True, tile_position=(HD, 0))
                            pt0 = pt_p.tile([P, 512], f32r, tag="pt")
                            pt1 = pt_p.tile([P, 512], f32r, tag="pt")
                            nc.scalar.activation(
                                pt0[:], pss0[:], mybir.ActivationFunctionType.Exp)
                            nc.scalar.activation(
                                pt1[:], pss1[:], mybir.ActivationFunctionType.Exp)
                            if kt >= 4 * J:
                                m = kt - 4 * J
                                nc.vector.tensor_mul(
                                    pt0[:], pt0[:], mask01[:, m, :])
                                nc.vector.tensor_mul(
                                    pt1[:], pt1[:], mask01[:, m, :])
                            nc.tensor.matmul(
                                psa0[:], vaug[:, kt, lkv * VW:(lkv + 1) * VW],
                                pt0[:],
                                start=(kt == 0), stop=(kt == nkt - 1),
                                skip_group_check=True)
                            nc.tensor.matmul(
                                psa1[:], vaug[:, kt, lkv * VW:(lkv + 1) * VW],
                                pt1[:],
                                start=(kt == 0), stop=(kt == nkt - 1),
                                skip_group_check=True)
                        for psa, poff in ((psa0, 0), (psa1, HD)):
                            rd = rd_p.tile([1, 512], f32, tag="rd")
                            nc.vector.reciprocal(rd[:], psa[HD:VW, :])
                            bc = bc_p.tile([HD, 512], f32, tag="bc")
                            nc.gpsimd.partition_broadcast(bc[:], rd[:])
                            nc.vector.tensor_mul(
                                aT[poff:poff + HD, t, :], psa[0:HD, :], bc[:])
                    for stl in range(4):
                        st = 4 * J + stl
                        for dn in range(D // 512):
                            psy = ps_y.tile([P, 512], f32, tag="psy")
                            for ft in range(FT):
                                nc.tensor.matmul(
                                    psy[:],
                                    aT[:, ft, stl * P:(stl + 1) * P],
                                    wo_sb[:, ft, dn * 512:(dn + 1) * 512],
                                    start=(ft == 0), stop=(ft == FT - 1))
                            yt = y_p.tile([P, 512], f32, tag="yt")
                            nc.scalar.copy(yt[:], psy[:])
                            nc.sync.dma_start(
                                out=out_t[:, st, dn * 512:(dn + 1) * 512],
                                in_=yt[:])
    nc.compile()
    return nc


def _host_tables(cos, sin, LQ, LKV, scale):
    # sign-folded rotate-half tables, replicated per head
    hd = cos.shape[1]
    sin_pm = np.concatenate([-sin[:, :hd // 2], sin[:, hd // 2:]], axis=1)
    cosq = np.tile(cos * scale, (1, LQ)).astype(np.float32)
    sinq = np.tile(sin_pm * scale, (1, LQ)).astype(np.float32)
    cosk = np.tile(cos, (1, LKV)).astype(np.float32)
    sink = np.tile(sin_pm, (1, LKV)).astype(np.float32)
    return cosq, sinq, cosk, sink




def prepare_in_maps(hidden_states, cos, sin, Wq, Wk, Wv, Wo, LQ=8, LKV=2):
    cosq, sinq, cosk, sink = _host_tables(cos, sin, LQ, LKV, SCALE)
    in_maps = []
    for c in range(8):
        b, g2 = c // 4, c % 4
        qs = g2 * LQ * HD
        ks = g2 * LKV * HD
        in_maps.append({
            "hs": np.ascontiguousarray(hidden_states[b]),
            "wq": np.ascontiguousarray(Wq[:, qs:qs + LQ * HD]),
            "wkv": np.ascontiguousarray(
                np.concatenate([Wk[:, ks:ks + LKV * HD],
                                Wv[:, ks:ks + LKV * HD]], axis=1)),
            "wo": np.ascontiguousarray(Wo[qs:qs + LQ * HD, :]),
            "cosq": cosq, "sinq": sinq, "cosk": cosk, "sink": sink,
        })
    return in_maps


_NC_CACHE = {}


def kernel(hidden_states, attention_mask, cos, sin, Wq, Wk, Wv, Wo):
    from concourse.bass_utils import run_bass_kernel_spmd

    hidden_states = np.asarray(hidden_states, dtype=np.float32)
    cos = np.asarray(cos, dtype=np.float32)
    sin = np.asarray(sin, dtype=np.float32)
    Wq = np.asarray(Wq, dtype=np.float32)
    Wk = np.asarray(Wk, dtype=np.float32)
    Wv = np.asarray(Wv, dtype=np.float32)
    Wo = np.asarray(Wo, dtype=np.float32)

    LQ, LKV = 8, 2
    if "nc" not in _NC_CACHE:
        _NC_CACHE["nc"] = build_nc(S, D, LQ, LKV, HD)
    nc = _NC_CACHE["nc"]

    in_maps = prepare_in_maps(hidden_states, cos, sin, Wq, Wk, Wv, Wo, LQ, LKV)
    res = run_bass_kernel_spmd(nc, in_maps, core_ids=list(range(8)))
    y = np.zeros((B, S, D), dtype=np.float32)
    for c in range(8):
        y[c // 4] += res.results[c]["out"]
    return y


# revision 9
# speedup vs baseline: 1.1948x; 1.1948x over previous
"""GQA attention (RoPE, causal) + o_proj on 8 Trainium2 NeuronCores.

Sharding: 8 cores = 2 batches (DP) x 4 kv-head-pairs (TP over GQA groups).
Per core: hsT[batch] [D,S] (host-pretransposed), Wq slice [D,512] (8 q heads),
Wk/Wv slice [D,128] (2 kv heads), Wo slice [512,D]. Core computes its heads'
attention and a partial o_proj output [S,D]; host sums 4 partials per batch.

Kernel dataflow (per core; fp32r matmuls, bf16 probabilities):
  1. q/k/v = hsT.T @ W (natural layout), fp32r matmuls
  2. RoPE in natural layout (host passes replicated/sign-folded trig tables)
  3. PE-transpose q,k -> qT [hd,s]; k2T has the kv head replicated on both
     64-partition halves for 2-head row-packed score matmuls
  4. scores^T[k,q] = k2T.T @ qT per 128k x 512q block, causal blocks only,
     two heads packed via tile_position row groups, two k-tiles per PSUM
     [128,1024] tile so exp amortizes ACT instruction overhead
  5. P^T = exp(scores^T) -> bf16 (ACT); 0/1 causal mask multiplied on diag
     blocks (DVE, bf16 4x)
  6. A^T_aug = [V|1].T @ P^T (bf16) accumulated over k tiles in PSUM; row 64
     is the softmax denominator; normalize via fast-reciprocal +
     gpsimd partition_broadcast + DVE multiply
  7. y = A @ Wo via lhsT=A^T (fp32r), accumulate 4 feature tiles, DMA out
"""
import sys
import numpy as np

sys.path.insert(0, "/opt/trn_rl_repo")

B, S, D = 2, 2048, 2048
H, KVH, HD = 32, 8, 64
SCALE = HD ** -0.5
P = 128


def build_nc(S=S, D=D, LQ=8, LKV=2, HD=64):
    import concourse.bacc as bacc
    import concourse.mybir as mybir
    from concourse import tile
    from concourse.masks import make_identity

    f32 = mybir.dt.float32
    f32r = mybir.dt.float32r
    bf16 = mybir.dt.bfloat16

    QF = LQ * HD          # local q features (512)
    KF = LKV * HD         # local kv features (128)
    FT = QF // P          # q feature tiles = head pairs (4)
    DT = D // P           # contraction tiles (16)
    ST = S // P           # sequence tiles (16)
    NJ = S // 512         # q supertiles (4)
    GRP = LQ // LKV       # q heads per kv head (4)
    VW = HD + 1           # v + ones column (65)
    Exp = mybir.ActivationFunctionType.Exp

    nc = bacc.Bacc(None, target_bir_lowering=False)
    hsT = nc.declare_dram_parameter("hsT", [D, S], f32r, isOutput=False)
    wq = nc.declare_dram_parameter("wq", [D, QF], f32r, isOutput=False)
    wkv = nc.declare_dram_parameter("wkv", [D, 2 * KF], f32r, isOutput=False)
    wo = nc.declare_dram_parameter("wo", [QF, D], f32r, isOutput=False)
    cosq = nc.declare_dram_parameter("cosq", [S, QF], f32, isOutput=False)
    sinq = nc.declare_dram_parameter("sinq", [S, QF], f32, isOutput=False)
    cosk = nc.declare_dram_parameter("cosk", [S, KF], f32, isOutput=False)
    sink = nc.declare_dram_parameter("sink", [S, KF], f32, isOutput=False)
    out = nc.declare_dram_parameter("out", [S, D], f32, isOutput=True)

    hsT_r = hsT.rearrange("(dt p) s -> p dt s", p=P)
    wq_t = wq.rearrange("(dt p) f -> p dt f", p=P)
    wkv_t = wkv.rearrange("(dt p) f -> p dt f", p=P)
    wo_t = wo.rearrange("(ft p) d -> p ft d", p=P)
    out_t = out.rearrange("(st p) d -> p st d", p=P)

    with tile.TileContext(nc) as tc:
        with tc.tile_pool(name="persist", bufs=1) as persist:
            ident = persist.tile([P, P], f32r)
            maskb = persist.tile([P, 4, 512], bf16)
            qT = persist.tile([P, FT, S], f32r)
            k2T = persist.tile([P, LKV, S], f32r)
            vaug = persist.tile([P, ST, LKV * VW], bf16)
            with tc.tile_pool(name="init", bufs=1) as init_p:
                ident_f = init_p.tile([P, P], f32)
                make_identity(nc, ident_f[:])
                nc.vector.tensor_copy(ident[:], ident_f[:])
                # maskb[kr, m, qc] = 1 where visible (qc >= 128*m+kr) else 0
                mask_f = init_p.tile([P, 4, 512], f32)
                for m in range(4):
                    nc.gpsimd.memset(mask_f[:, m, :], 1.0)
                    nc.gpsimd.affine_select(
                        out=mask_f[:, m, :], in_=mask_f[:, m, :],
                        compare_op=mybir.AluOpType.is_ge, fill=0.0,
                        base=-P * m, pattern=[[1, 512]], channel_multiplier=-1,
                    )
                nc.vector.tensor_copy(maskb[:], mask_f[:])
                ones_f = init_p.tile([P, 1], f32)
                nc.gpsimd.memset(ones_f[:], 1.0)
                for lkv in range(LKV):
                    for st_i in range(ST):
                        nc.vector.tensor_copy(
                            vaug[:, st_i, lkv * VW + HD:lkv * VW + HD + 1],
                            ones_f[:])

            # ---------------- phase 1: projections + RoPE + transposes ----
            with (
                tc.tile_pool(name="wq_p", bufs=1) as wq_p,
                tc.tile_pool(name="wkv_p", bufs=1) as wkv_p,
                tc.tile_pool(name="hsT", bufs=2) as hsT_p,
                tc.tile_pool(name="trig", bufs=2) as trig_p,
                tc.tile_pool(name="rope", bufs=2) as rope_p,
                tc.tile_pool(name="ps_tr", bufs=4, space="PSUM") as ps_tr,
                tc.tile_pool(name="ps_q", bufs=2, space="PSUM") as ps_q,
                tc.tile_pool(name="ps_kv", bufs=2, space="PSUM") as ps_kv,
            ):
                wq_sb = wq_p.tile([P, DT, QF], f32r)
                nc.sync.dma_start(out=wq_sb[:], in_=wq_t)
                wkv_sb = wkv_p.tile([P, DT, 2 * KF], f32r)
                nc.sync.dma_start(out=wkv_sb[:], in_=wkv_t)

                for blk in range(S // 256):
                    hsT_b = hsT_p.tile([P, DT, 256], f32r, tag="hsT")
                    nc.sync.dma_start(
                        out=hsT_b[:],
                        in_=hsT_r[:, :, blk * 256:(blk + 1) * 256])
                    for j in range(2):
                        si = blk * 2 + j
                        # ---- q = hs @ Wq (natural), RoPE, transpose ----
                        psq = ps_q.tile([P, QF], f32, tag="psq")
                        for dt in range(DT):
                            nc.tensor.matmul(
                                psq[:], hsT_b[:, dt, j * P:(j + 1) * P],
                                wq_sb[:, dt, :],
                                start=(dt == 0), stop=(dt == DT - 1))
                        cq = trig_p.tile([P, QF], f32, tag="cq")
                        nc.sync.dma_start(
                            out=cq[:], in_=cosq[si * P:(si + 1) * P, :])
                        sq = trig_p.tile([P, QF], f32, tag="sq")
                        nc.sync.dma_start(
                            out=sq[:], in_=sinq[si * P:(si + 1) * P, :])
                        qrot = rope_p.tile([P, QF], f32r, tag="qrot")
                        qtmp = rope_p.tile([P, QF], f32, tag="qtmp")
                        rsh = rope_p.tile([P, QF], f32, tag="rsh")
                        psq_v = psq[:].rearrange("p (h t x) -> p h t x", t=2, x=32)
                        rsh_v = rsh[:].rearrange("p (h t x) -> p h t x", t=2, x=32)
                        nc.vector.tensor_copy(rsh_v[:, :, 0, :], psq_v[:, :, 1, :])
                        nc.vector.tensor_copy(rsh_v[:, :, 1, :], psq_v[:, :, 0, :])
                        nc.vector.tensor_mul(qtmp[:], psq[:], cq[:])
                        nc.vector.tensor_mul(rsh[:], rsh[:], sq[:])
                        nc.vector.tensor_add(qrot[:], qtmp[:], rsh[:])
                        for ft in range(FT):
                            pt = ps_tr.tile([P, P], f32r, tag="ptr")
                            nc.tensor.transpose(
                                pt[:], qrot[:, ft * P:(ft + 1) * P], ident[:])
                            nc.scalar.copy(qT[:, ft, si * P:(si + 1) * P], pt[:])
                        # ---- k/v ----
                        pskv = ps_kv.tile([P, 2 * KF], f32, tag="pskv")
                        for dt in range(DT):
                            nc.tensor.matmul(
                                pskv[:], hsT_b[:, dt, j * P:(j + 1) * P],
                                wkv_sb[:, dt, :],
                                start=(dt == 0), stop=(dt == DT - 1))
                        ck = trig_p.tile([P, KF], f32, tag="ck")
                        nc.sync.dma_start(
                            out=ck[:], in_=cosk[si * P:(si + 1) * P, :])
                        sk = trig_p.tile([P, KF], f32, tag="sk")
                        nc.sync.dma_start(
                            out=sk[:], in_=sink[si * P:(si + 1) * P, :])
                        krot = rope_p.tile([P, KF], f32r, tag="krot")
                        ktmp = rope_p.tile([P, KF], f32, tag="ktmp")
                        krsh = rope_p.tile([P, KF], f32, tag="krsh")
                        psk_v = pskv[:, 0:KF].rearrange(
                            "p (h t x) -> p h t x", t=2, x=32)
                        krsh_v = krsh[:].rearrange("p (h t x) -> p h t x", t=2, x=32)
                        nc.vector.tensor_copy(krsh_v[:, :, 0, :], psk_v[:, :, 1, :])
                        nc.vector.tensor_copy(krsh_v[:, :, 1, :], psk_v[:, :, 0, :])
                        nc.vector.tensor_mul(ktmp[:], pskv[:, 0:KF], ck[:])
                        nc.vector.tensor_mul(krsh[:], krsh[:], sk[:])
                        nc.vector.tensor_add(krot[:], ktmp[:], krsh[:])
                        for lkv in range(LKV):
                            pt = ps_tr.tile([P, P], f32r, tag="ptr")
                            nc.tensor.transpose(
                                pt[0:HD, :],
                                krot[:, lkv * HD:(lkv + 1) * HD], ident[:])
                            nc.scalar.copy(
                                k2T[0:HD, lkv, si * P:(si + 1) * P], pt[0:HD, :])
                            nc.scalar.copy(
                                k2T[HD:P, lkv, si * P:(si + 1) * P], pt[0:HD, :])
                            nc.scalar.copy(
                                vaug[:, si, lkv * VW:lkv * VW + HD],
                                pskv[:, KF + lkv * HD:KF + (lkv + 1) * HD])

            # ---------------- phase 2+3: attention + o_proj ---------------
            with (
                tc.tile_pool(name="wo_p", bufs=1) as wo_p,
                tc.tile_pool(name="pt_p", bufs=20) as pt_p,
                tc.tile_pool(name="aT_p", bufs=2) as aT_p,
                tc.tile_pool(name="bc_p", bufs=4) as bc_p,
                tc.tile_pool(name="rd_p", bufs=4) as rd_p,
                tc.tile_pool(name="y_p", bufs=3) as y_p,
                tc.tile_pool(name="ps_s", bufs=2, space="PSUM") as ps_s,
                tc.tile_pool(name="ps_a", bufs=2, space="PSUM") as ps_a,
                tc.tile_pool(name="ps_y", bufs=2, space="PSUM") as ps_y,
            ):
                wo_sb = wo_p.tile([P, FT, D], f32r)
                nc.sync.dma_start(out=wo_sb[:], in_=wo_t)

                for J in range(NJ):
                    aT = aT_p.tile([P, FT, 512], f32r, tag="aT")
                    nkt = 4 * J + 4
                    for t in range(FT):          # head pair (2t, 2t+1)
                        lkv = (2 * t) // GRP
                        psa0 = ps_a.tile([VW, 512], f32, tag="psa")
                        psa1 = ps_a.tile([VW, 512], f32, tag="psa")
                        pts = []
                        for kp in range(nkt // 2):
                            pss0 = ps_s.tile([P, 1024], f32, tag="pss")
                            pss1 = ps_s.tile([P, 1024], f32, tag="pss")
                            for i in range(2):
                                kt = 2 * kp + i
                                nc.tensor.matmul(
                                    pss0[:, i * 512:(i + 1) * 512],
                                    k2T[0:HD, lkv, kt * P:(kt + 1) * P],
                                    qT[0:HD, t, J * 512:(J + 1) * 512],
                                    start=True, stop=True,
                                    tile_position=(0, 0))
                                nc.tensor.matmul(
                                    pss1[:, i * 512:(i + 1) * 512],
                                    k2T[HD:P, lkv, kt * P:(kt + 1) * P],
                                    qT[HD:P, t, J * 512:(J + 1) * 512],
                                    start=True, stop=True,
                                    tile_position=(HD, 0))
                            pt0 = pt_p.tile([P, 1024], bf16, tag="pt")
                            pt1 = pt_p.tile([P, 1024], bf16, tag="pt")
                            nc.scalar.activation(pt0[:], pss0[:], Exp)
                            nc.scalar.activation(pt1[:], pss1[:], Exp)
                            for i in range(2):
                                kt = 2 * kp + i
                                if kt >= 4 * J:
                                    m = kt - 4 * J
                                    nc.vector.tensor_mul(
                                        pt0[:, i * 512:(i + 1) * 512],
                                        pt0[:, i * 512:(i + 1) * 512],
                                        maskb[:, m, :])
                                    nc.vector.tensor_mul(
                                        pt1[:, i * 512:(i + 1) * 512],
                                        pt1[:, i * 512:(i + 1) * 512],
                                        maskb[:, m, :])
                            pts.append((pt0, pt1))
                        for kt in range(nkt):
                            pt0, pt1 = pts[kt // 2]
                            i = kt % 2
                            nc.tensor.matmul(
                                psa0[:], vaug[:, kt, lkv * VW:(lkv + 1) * VW],
                                pt0[:, i * 512:(i + 1) * 512],
                                start=(kt == 0), stop=(kt == nkt - 1),
                                skip_group_check=True)
                            nc.tensor.matmul(
                                psa1[:], vaug[:, kt, lkv * VW:(lkv + 1) * VW],
                                pt1[:, i * 512:(i + 1) * 512],
                                start=(kt == 0), stop=(kt == nkt - 1),
                                skip_group_check=True)
                        for psa, poff in ((psa0, 0), (psa1, HD)):
                            dn = rd_p.tile([1, 512], f32, tag="dn")
                            nc.vector.tensor_copy(dn[:], psa[HD:VW, :])
                            rc = rd_p.tile([1, 512], f32, tag="rc")
                            nc.vector.reciprocal_approx_fast(rc[:], dn[:])
                            dnb = bc_p.tile([HD, 512], f32, tag="bc")
                            nc.gpsimd.partition_broadcast(dnb[:], rc[:])
                            nc.vector.tensor_mul(
                                aT[poff:poff + HD, t, :], psa[0:HD, :], dnb[:])
                    for stl in range(4):
                        st = 4 * J + stl
                        for dn_i in range(D // 512):
                            psy = ps_y.tile([P, 512], f32, tag="psy")
                            for ft in range(FT):
                                nc.tensor.matmul(
                                    psy[:],
                                    aT[:, ft, stl * P:(stl + 1) * P],
                                    wo_sb[:, ft, dn_i * 512:(dn_i + 1) * 512],
                                    start=(ft == 0), stop=(ft == FT - 1))
                            yt = y_p.tile([P, 512], f32, tag="yt")
                            nc.vector.tensor_copy(yt[:], psy[:])
                            nc.sync.dma_start(
                                out=out_t[:, st, dn_i * 512:(dn_i + 1) * 512],
                                in_=yt[:])
    nc.compile()
    return nc


def _host_tables(cos, sin, LQ, LKV, scale):
    # sign-folded rotate-half tables, replicated per head
    hd = cos.shape[1]
    sin_pm = np.concatenate([-sin[:, :hd // 2], sin[:, hd // 2:]], axis=1)
    cosq = np.tile(cos * scale, (1, LQ)).astype(np.float32)
    sinq = np.tile(sin_pm * scale, (1, LQ)).astype(np.float32)
    cosk = np.tile(cos, (1, LKV)).astype(np.float32)
    sink = np.tile(sin_pm, (1, LKV)).astype(np.float32)
    return cosq, sinq, cosk, sink


def prepare_in_maps(hidden_states, cos, sin, Wq, Wk, Wv, Wo, LQ=8, LKV=2):
    cosq, sinq, cosk, sink = _host_tables(cos, sin, LQ, LKV, SCALE)
    nb = hidden_states.shape[0]
    hsT = [np.ascontiguousarray(hidden_states[b].T) for b in range(nb)]
    in_maps = []
    for c in range(8):
        b, g2 = c // 4, c % 4
        qs = g2 * LQ * HD
        ks = g2 * LKV * HD
        in_maps.append({
            "hsT": hsT[b],
            "wq": np.ascontiguousarray(Wq[:, qs:qs + LQ * HD]),
            "wkv": np.ascontiguousarray(
                np.concatenate([Wk[:, ks:ks + LKV * HD],
                                Wv[:, ks:ks + LKV * HD]], axis=1)),
            "wo": np.ascontiguousarray(Wo[qs:qs + LQ * HD, :]),
            "cosq": cosq, "sinq": sinq, "cosk": cosk, "sink": sink,
        })
    return in_maps


_NC_CACHE = {}


def kernel(hidden_states, attention_mask, cos, sin, Wq, Wk, Wv, Wo):
    from concourse.bass_utils import run_bass_kernel_spmd

    hidden_states = np.asarray(hidden_states, dtype=np.float32)
    cos = np.asarray(cos, dtype=np.float32)
    sin = np.asarray(sin, dtype=np.float32)
    Wq = np.asarray(Wq, dtype=np.float32)
    Wk = np.asarray(Wk, dtype=np.float32)
    Wv = np.asarray(Wv, dtype=np.float32)
    Wo = np.asarray(Wo, dtype=np.float32)

    LQ, LKV = 8, 2
    if "nc" not in _NC_CACHE:
        _NC_CACHE["nc"] = build_nc(S, D, LQ, LKV, HD)
    nc = _NC_CACHE["nc"]

    in_maps = prepare_in_maps(hidden_states, cos, sin, Wq, Wk, Wv, Wo, LQ, LKV)
    res = run_bass_kernel_spmd(nc, in_maps, core_ids=list(range(8)))
    y = np.zeros((B, S, D), dtype=np.float32)
    for c in range(8):
        y[c // 4] += res.results[c]["out"]
    return y
